# revision 4
# baseline (speedup 1.0000x reference)
"""KPConv (nn_KPConvFPN) Trainium2 Bass kernel, v3.

Sharding: 8 cores; core c handles batch b=c//2, query half (c%2)*8192.
Host packs a 256B/row table: [64 x fp16 feat | sx,sy,sz fp32 | z fp32 | pad].
Device pipeline per core:
  1. dma_gather (SWDGE, 4 queues) of the 16 neighbor rows per query:
     1024 idx/chunk, idx order (query-block, k) -> partitions (q8,k16).
  2. kw[n,k,p] = relu(1 - sqrt(|s - q - kp_p|^2)/sigma) via DVE/ACT,
     output fp16.
  3. einsum1 on PE per 8-query block: lhsT = gathered f (fp16), rhs =
     kw * blockdiag-mask (fp16) -> weightedT [c=64, (q',p)] in PSUM.
  4. einsum2 on PE in fp16: lhsT = W_p [64, 128] , rhs = weightedT
     strided -> out [o=128, n] accumulated over p.
  5. count[n] = sum_k z via masked-z ones-row matmul (fp16); fast
     reciprocal; divide + bias, PE-transpose to [n, o], store.
"""
import json
import math
import os

SKIP = set()

import numpy as np
import jax

import concourse.bass as bass
import concourse.mybir as mybir
from concourse.tile import TileContext
from concourse import bass2jax

F32 = mybir.dt.float32
F16 = mybir.dt.float16
I32 = mybir.dt.int32
I16 = mybir.dt.int16

B, N, M, K = 4, 16384, 16384, 16
C_IN, C_OUT, P = 64, 128, 15
SIGMA = 0.03
N_CORES = 8
NQ_CORE = N // 2            # 8192 queries per core
NK_CORE = NQ_CORE * K       # 131072 gathered rows per core
ST_Q = 512                  # queries per supertile
N_ST = NQ_CORE // ST_Q      # 16
KW_ST = 2                   # supertiles per kw group
G_ST = ST_Q * K // 128      # 64 g-cols per supertile
ROW16 = 128                 # f16 units per table row (256B)

# ---------------------------------------------------------------------------
# walrus workaround: this nix walrus build supports ONE sync-wait per
# instruction; split extra waits onto NoOps inserted before the offender
# (same-engine program order preserves semantics). Also run
# codegen_inst_isa_subclasses (Bacc does; raw Bass doesn't) so extended
# instructions get their ISA bytes.
_orig_to_json_bytes = bass.Bass.to_json_bytes


def _fix_block(bb, ctr):
    insts = bb.get("instructions")
    if not isinstance(insts, list):
        return
    new = []
    for inst in insts:
        si = inst.get("sync_info")
        ow = si.get("on_wait") if isinstance(si, dict) else None
        if ow and len(ow) > 1:
            for w in ow[:-1]:
                ctr[0] += 1
                nop = {"engine": inst["engine"], "ins": [], "outs": [],
                       "name": f"I-wsplit-{ctr[0]}", "opcode": "NoOp",
                       "sync_info": {"on_update": [], "on_wait": [w]},
                       "text_hint": "wsplit"}
                if "debug" in inst:
                    nop["debug"] = inst["debug"]
                new.append(nop)
            si["on_wait"] = [ow[-1]]
        new.append(inst)
    bb["instructions"] = new


def _walk(o, ctr):
    if isinstance(o, dict):
        if isinstance(o.get("instructions"), list):
            _fix_block(o, ctr)
        for v in o.values():
            _walk(v, ctr)
    elif isinstance(o, list):
        for v in o:
            _walk(v, ctr)


def _to_json_bytes_split(self):
    mybir.codegen_inst_isa_subclasses(self)
    raw = _orig_to_json_bytes(self)
    d = json.loads(raw)
    ctr = [0]
    _walk(d, ctr)
    return json.dumps(d).encode()


bass.Bass.to_json_bytes = _to_json_bytes_split


def ap_view(t_ap, extra_offset, dims):
    """AP over tile t_ap with explicit free dims [[step, count], ...]
    (steps in elements); partition dim is taken from the tile."""
    return bass.AP(t_ap.tensor, t_ap.offset + extra_offset,
                   [t_ap.ap[0]] + list(dims))


def build_bass(kp, skip=()):
    global SKIP
    SKIP = set(skip)
    """kp: (15, 3) float32 numpy kernel points (runtime values baked)."""
    nc = bass.Bass(dynamic_dma_scratch_size=49152, num_swdge_queues=4)

    table_in = nc.dram_tensor("table", [M, ROW16], F16, kind="ExternalInput")
    qrep_in = nc.dram_tensor("qrep", [128, NK_CORE // 128, 3], F32,
                             kind="ExternalInput")
    idx_in = nc.dram_tensor("idx", [128, NK_CORE // 16], I16,
                            kind="ExternalInput")
    from concourse import library_config
    nc.gpsimd.load_library(library_config.mlp)
    w_in = nc.dram_tensor("w", [C_IN, P * C_OUT], F16, kind="ExternalInput")
    bias_in = nc.dram_tensor("bias", [C_OUT, 1], F32, kind="ExternalInput")
    mask120_in = nc.dram_tensor("mask120", [128, 120], F16, kind="ExternalInput")
    mask16_in = nc.dram_tensor("mask16", [128, 8], F16, kind="ExternalInput")
    ident_in = nc.dram_tensor("ident", [128, 128], F32, kind="ExternalInput")
    ones1_in = nc.dram_tensor("ones1", [1, 128], F16, kind="ExternalInput")
    kpb_in = nc.dram_tensor("kpb", [128, 48], F32, kind="ExternalInput")
    kpb3_in = nc.dram_tensor("kpb3", [128, 45], F16, kind="ExternalInput")
    onesc_in = nc.dram_tensor("onesc", [128, 1], F16, kind="ExternalInput")
    out_t = nc.dram_tensor("out", [NQ_CORE, C_OUT], F32, kind="ExternalOutput")

    with TileContext(nc) as tc:
        with tc.tile_pool(name="const", bufs=1) as cpool, \
             tc.tile_pool(name="gath", bufs=2) as gpool, \
             tc.tile_pool(name="kwp", bufs=2) as kwpool, \
             tc.tile_pool(name="kbd", bufs=1) as kbpool, \
             tc.tile_pool(name="wt", bufs=1) as wtpool, \
             tc.tile_pool(name="sm", bufs=3) as smpool, \
             tc.tile_pool(name="fin", bufs=2) as fpool, \
             tc.tile_pool(name="ps1", bufs=2, space="PSUM") as ps1pool, \
             tc.tile_pool(name="ps2", bufs=2, space="PSUM") as ps2pool, \
             tc.tile_pool(name="ps3", bufs=1, space="PSUM") as ps3pool:

            # ---- constants ----
            wp_t = cpool.tile([C_IN, P * C_OUT], F16, tag="wp")
            nc.sync.dma_start(wp_t[:], w_in[:])
            bias_t = cpool.tile([C_OUT, 1], F32, tag="bias")
            nc.sync.dma_start(bias_t[:], bias_in[:])
            mask120_t = cpool.tile([128, 120], F16, tag="m120")
            nc.sync.dma_start(mask120_t[:], mask120_in[:])
            mask16_t = cpool.tile([128, 8], F16, tag="m16")
            nc.sync.dma_start(mask16_t[:], mask16_in[:])
            ident_t = cpool.tile([128, 128], F32, tag="ident")
            nc.sync.dma_start(ident_t[:], ident_in[:])
            ones1_t = cpool.tile([1, 128], F16, tag="ones1")
            nc.sync.dma_start(ones1_t[:], ones1_in[:])
            kpb_t = cpool.tile([128, 48], F32, tag="kpb")
            nc.sync.dma_start(kpb_t[:], kpb_in[:])
            kpb3_t = cpool.tile([128, 45], F16, tag="kpb3")
            nc.sync.dma_start(kpb3_t[:], kpb3_in[:])
            onesc_t = cpool.tile([128, 1], F16, tag="onesc")
            nc.sync.dma_start(onesc_t[:], onesc_in[:])
            nidx_reg = nc.gpsimd.to_reg(1024)

            _main_pipeline(nc, tc, gpool, kwpool, kbpool, wtpool, smpool,
                           fpool, ps1pool, ps2pool, ps3pool, kp,
                           qrep_in, idx_in, out_t, table_in, wp_t, bias_t,
                           mask120_t, mask16_t, ident_t, ones1_t, kpb_t,
                           onesc_t, kpb3_t, nidx_reg)
    return nc


def _main_pipeline(nc, tc, gpool, kwpool, kbpool, wtpool, smpool, fpool,
                   ps1pool, ps2pool, ps3pool, kp, qrep_in, idx_in, out_t,
                   table_in, wp_t, bias_t, mask120_t, mask16_t, ident_t,
                   ones1_t, kpb_t, onesc_t, kpb3_t, nidx_reg):
    for kg in range(N_ST // KW_ST):  # kw group of 2 supertiles
        GQ = KW_ST * ST_Q            # 1024 queries
        GG = KW_ST * G_ST            # 128 g-cols
        gt = gpool.tile([128, GG, ROW16], F16, tag="gath")
        gt32 = gt[:].bitcast(F32)  # [128, GG, 64] f32 view
        # gathers: 16 chunks of 1024 idx
        if "gather" in SKIP:
            nc.vector.memset(gt[:], 0.0)
        for g in range(GG // 8):
            if "gather" in SKIP:
                break
            idxsl = smpool.tile([128, 64], I16, tag="idxsl")
            nc.sync.dma_start(
                idxsl[:],
                idx_in[:, (kg * 16 + g) * 64:(kg * 16 + g) * 64 + 64])
            nc.gpsimd.dma_gather(
                gt[:, g * 8:(g + 1) * 8, :], table_in[:], idxsl[:],
                1024, nidx_reg, ROW16, queue_num=g % 4)
        # qrep slice
        qr = smpool.tile([128, GG, 3], F32, tag="qr")
        nc.sync.dma_start(qr[:], qrep_in[:, kg * GG:(kg + 1) * GG, :])
        # rel = s - q (fp16)
        rel = smpool.tile([128, GG, 3], F16, tag="rel")
        nc.vector.tensor_tensor(
            out=rel[:],
            in0=ap_view(gt32, 32, [[64, GG], [1, 3]]),
            in1=qr[:], op=mybir.AluOpType.subtract)
        # diff[g,p,d] = rel[g,d] - kp[p,d]; square; sum over d; sqrt; relu
        kwt = kwpool.tile([128, GG, P], F16, tag="kw")
        if "kw" in SKIP:
            nc.vector.memset(kwt[:], 0.0)
        else:
            diff = kwpool.tile([128, GG, P, 3], F16, tag="diff")
            nc.vector.tensor_tensor(
                out=diff[:],
                in0=ap_view(rel[:], 0, [[3, GG], [0, P], [1, 3]]),
                in1=ap_view(kpb3_t[:], 0, [[0, GG], [3, P], [1, 3]]),
                op=mybir.AluOpType.subtract)
            nc.scalar.activation(diff[:], diff[:],
                                 mybir.ActivationFunctionType.Square,
                                 bias=0.0, scale=1.0)
            d2 = kwpool.tile([128, GG, P], F16, tag="d2")
            with nc.allow_low_precision(reason="d2 sum of 3 sq in fp16"):
                nc.vector.tensor_reduce(out=d2[:], in_=diff[:],
                                        axis=mybir.AxisListType.X,
                                        op=mybir.AluOpType.add)
            # kw = relu(1 - sqrt(d2)/sigma) -> fp16
            nc.scalar.activation(d2[:], d2[:],
                                 mybir.ActivationFunctionType.Sqrt,
                                 bias=0.0, scale=1.0)
            nc.scalar.activation(kwt[:], d2[:],
                                 mybir.ActivationFunctionType.Relu,
                                 bias=1.0, scale=kpb_t[:, 46:47])

        for sti in range(KW_ST):
            st = kg * KW_ST + sti
            # kwbd (2 half-ST TT ops): [128, (bl32, q8, p15)] fp16
            kbd = kbpool.tile([128, 3840], F16, tag="kbd")
            kbd2 = kbpool.tile([128, 3840], F16, tag="kbd2")
            if "kwbd" in SKIP:
                nc.vector.memset(kbd[:], 0.0)
                nc.vector.memset(kbd2[:], 0.0)
            for hf, kb in ((0, kbd), (1, kbd2)) if "kwbd" not in SKIP else ():
                bl0 = sti * G_ST + hf * 32
                nc.vector.tensor_tensor(
                    out=ap_view(kb[:], 0,
                                [[120, 32], [15, 8], [1, 15]]),
                    in0=ap_view(kwt[:], bl0 * P,
                                [[P, 32], [0, 8], [1, P]]),
                    in1=ap_view(mask120_t[:], 0,
                                [[0, 32], [15, 8], [1, 15]]),
                    op=mybir.AluOpType.mult)
            # einsum1: 64 blocks
            wtt = wtpool.tile([64, 7680], F16, tag="wt")
            if "e1" in SKIP:
                nc.vector.memset(wtt[:], 0.0)
            for bg in range(16 if "e1" not in SKIP else 0):  # bank groups of 4 blocks (32 q)
                pse1 = ps1pool.tile([64, 480], F32, tag="pse1")
                for j in range(4):
                    bl = bg * 4 + j          # block in supertile
                    blg = sti * G_ST + bl    # g-col in group tile
                    kb = kbd if bl < 32 else kbd2
                    kbl = bl % 32
                    nc.tensor.matmul(
                        pse1[:, j * 120:(j + 1) * 120],
                        ap_view(gt[:], blg * ROW16, [[1, C_IN]]),
                        ap_view(kb[:], kbl * 120, [[1, 120]]),
                        start=True, stop=True)
                # evict (split DVE/ACT) -> fp16
                nc.vector.tensor_copy(
                    wtt[:, bg * 480:bg * 480 + 240],
                    pse1[:, 0:240])
                nc.scalar.copy(
                    wtt[:, bg * 480 + 240:bg * 480 + 480],
                    pse1[:, 240:480])
            # count row: zbd = z * mask16 -> ones-row matmul (fp16)
            zbd = smpool.tile([128, 512], F16, tag="zbd")
            nc.vector.tensor_tensor(
                out=zbd[:].rearrange("a (g j q) -> a g j q",
                                     g=16, j=4),
                in0=ap_view(gt32, (sti * G_ST) * 64 + 35,
                            [[256, 16], [64, 4], [0, 8]]),
                in1=ap_view(mask16_t[:], 0,
                            [[0, 16], [0, 4], [1, 8]]),
                op=mybir.AluOpType.mult)
            pscnt = ps3pool.tile([1, 512], F32, tag="pscnt")
            nc.tensor.matmul(pscnt[:], onesc_t[:], zbd[:],
                             start=True, stop=True)
            cntinv = smpool.tile([1, 512], F32, tag="cntinv")
            nc.vector.tensor_scalar(out=cntinv[:], in0=pscnt[:],
                                    scalar1=1.0, scalar2=None,
                                    op0=mybir.AluOpType.max)
            rscr = smpool.tile([1, 512], F32, tag="rscr")
            nc.vector.reciprocal_approx_accurate(out=cntinv[:], in_=cntinv[:],
                                                 scratch=rscr[:])
            cntinv16 = smpool.tile([1, 512], F16, tag="cntinv16")
            nc.vector.tensor_copy(cntinv16[:], cntinv[:])
            psrep = ps3pool.tile([128, 512], F32, tag="psrep")
            nc.tensor.matmul(psrep[:], ones1_t[:], cntinv16[:],
                             start=True, stop=True)
            cntrep = smpool.tile([128, 512], F32, tag="cntrep")
            nc.vector.tensor_copy(cntrep[:], psrep[:])

            # einsum2: out[o, s] accumulated over p (fp16 operands)
            pse2 = ps2pool.tile([128, 512], F32, tag="pse2")
            for p in range(P if "e2" not in SKIP else 1):
                nc.tensor.matmul(
                    pse2[:],
                    ap_view(wp_t[:], p * C_OUT, [[1, C_OUT]]),
                    ap_view(wtt[:], p,
                            [[480, 16], [120, 4], [15, 8]]),
                    start=(p == 0), stop=True)
            # divide by count, add bias
            e2sb = fpool.tile([128, 512], F32, tag="e2sb")
            nc.vector.tensor_tensor(out=e2sb[:], in0=pse2[:],
                                    in1=cntrep[:],
                                    op=mybir.AluOpType.mult)
            nc.vector.tensor_scalar(out=e2sb[:], in0=e2sb[:],
                                    scalar1=bias_t[:],
                                    scalar2=None,
                                    op0=mybir.AluOpType.add)
            # transpose 4x128 cols and store
            for t4 in range(4):
                pstr = ps3pool.tile([128, 128], F32, tag="pstr")
                nc.tensor.transpose(
                    pstr[:], e2sb[:, t4 * 128:(t4 + 1) * 128],
                    ident_t[:])
                trsb = fpool.tile([128, 128], F32, tag="trsb")
                nc.scalar.copy(trsb[:], pstr[:])
                # e2 cols are n-linear: plain contiguous store
                n0 = st * 512 + t4 * 128
                nc.sync.dma_start(out_t[n0:n0 + 128, :], trsb[:])


_BUILT = {}


def _get_nc(kp):
    key = kp.tobytes()
    if key not in _BUILT:
        _BUILT[key] = build_bass(kp)
    return _BUILT[key]


def _host_prep(query_points, support_points, support_features,
               neighbor_indices, weights, bias, kernel_points):
    qp = np.asarray(query_points, np.float32)
    sp = np.asarray(support_points, np.float32)
    sf = np.asarray(support_features, np.float32)
    ni = np.asarray(neighbor_indices)
    ni = np.clip(ni, 0, M - 1).astype(np.int16)
    w = np.ascontiguousarray(np.asarray(weights, np.float32))
    # w layout [C_IN, P*C_OUT] fp16: wl[c, p*C_OUT + o] = w[p, c, o]
    wl = np.ascontiguousarray(
        w.transpose(1, 0, 2).reshape(C_IN, P * C_OUT)).astype(np.float16)
    bias = np.asarray(bias, np.float32).reshape(C_OUT, 1)

    mask120 = np.zeros((128, 120), np.float16)
    for q in range(8):
        mask120[q * 16:(q + 1) * 16, q * 15:(q + 1) * 15] = 1.0
    mask16 = np.zeros((128, 8), np.float16)
    for q in range(8):
        mask16[q * 16:(q + 1) * 16, q] = 1.0
    ident = np.eye(128, dtype=np.float32)
    ones1 = np.ones((1, 128), np.float16)
    kpv = np.asarray(kernel_points, np.float32)
    kpb = np.zeros((128, 48), np.float32)
    for p in range(P):
        for d in range(3):
            kpb[:, 3 * p + d] = -kpv[p, d]
    kpb[:, 45] = 1e-10
    kpb[:, 46] = -1.0 / SIGMA
    kpb3 = np.tile(kpv.reshape(1, 45), (128, 1)).astype(np.float16)

    # host-built tables per batch: [M, ROW16] f16 rows (256B)
    # f16 cols 0..63 = feats; f32-view cols 32..34 = coords, 35 = z
    tables = []
    for b in range(B):
        t = np.zeros((M, ROW16), np.float16)
        tv32 = t.view(np.float32)  # [M, 64]
        t[:, 0:C_IN] = sf[b].astype(np.float16)
        tv32[:, 32:35] = sp[b]
        tv32[:, 35] = (np.abs(sf[b]).sum(axis=1) > 0).astype(np.float32)
        tables.append(t)

    in_maps = []
    for c in range(N_CORES):
        b, half = divmod(c, 2)
        n0 = half * NQ_CORE
        idx = ni[b, n0:n0 + NQ_CORE, :].reshape(NK_CORE)
        # chunk order: idx j in chunk -> partition j%128, col j//128;
        # idx tile wraps 16 partitions, replicated x8
        idx_l = idx.reshape(NK_CORE // 16, 16).T          # [16, NK/16]
        idx_l = np.ascontiguousarray(np.tile(idx_l, (8, 1)).astype(np.int16))
        qrep = np.repeat(qp[b, n0:n0 + NQ_CORE, :], K, axis=0)  # [NK, 3]
        qrep = qrep.reshape(NK_CORE // 128, 128, 3).transpose(1, 0, 2)
        qrep = np.ascontiguousarray(qrep)
        in_maps.append({
            "table": tables[b], "qrep": qrep, "idx": idx_l,
            "w": wl, "bias": bias, "mask120": mask120, "mask16": mask16,
            "ident": ident, "ones1": ones1, "kpb": kpb, "kpb3": kpb3,
            "onesc": np.ones((128, 1), np.float16),
        })
    return in_maps


def kernel(query_points, support_points, support_features, neighbor_indices,
           weights, bias, kernel_points):
    kp = np.asarray(kernel_points, np.float32)
    nc = _get_nc(kp)
    in_maps = _host_prep(query_points, support_points, support_features,
                         neighbor_indices, weights, bias, kernel_points)
    results = bass2jax.run_bass_via_pjrt(nc, in_maps, n_cores=N_CORES)
    out = np.zeros((B, N, C_OUT), np.float32)
    for c in range(N_CORES):
        b, half = divmod(c, 2)
        n0 = half * NQ_CORE
        out[b, n0:n0 + NQ_CORE, :] = np.asarray(results[c]["out"])
    return out


# revision 7
# speedup vs baseline: 1.2885x; 1.2885x over previous
"""KPConv (nn_KPConvFPN) Trainium2 Bass kernel, v3.

Sharding: 8 cores; core c handles batch b=c//2, query half (c%2)*8192.
Host packs a 256B/row table: [64 x fp16 feat | sx,sy,sz fp32 | z fp32 | pad].
Device pipeline per core:
  1. dma_gather (SWDGE, 4 queues) of the 16 neighbor rows per query:
     1024 idx/chunk, idx order (query-block, k) -> partitions (q8,k16).
  2. kw[n,k,p] = relu(1 - sqrt(|s - q - kp_p|^2)/sigma) via DVE/ACT,
     output fp16.
  3. einsum1 on PE per 8-query block: lhsT = gathered f (fp16), rhs =
     kw * blockdiag-mask (fp16) -> weightedT [c=64, (q',p)] in PSUM.
  4. einsum2 on PE in fp16: lhsT = W_p [64, 128] , rhs = weightedT
     strided -> out [o=128, n] accumulated over p.
  5. count[n] = sum_k z via masked-z ones-row matmul (fp16); fast
     reciprocal; divide + bias, PE-transpose to [n, o], store.
"""
import json
import math
import os

SKIP = set()

import numpy as np
import jax

import concourse.bass as bass
import concourse.mybir as mybir
from concourse.tile import TileContext
from concourse import bass2jax

F32 = mybir.dt.float32
F16 = mybir.dt.float16
I32 = mybir.dt.int32
I16 = mybir.dt.int16

B, N, M, K = 4, 16384, 16384, 16
C_IN, C_OUT, P = 64, 128, 15
SIGMA = 0.03
N_CORES = 8
NQ_CORE = N // 2            # 8192 queries per core
NK_CORE = NQ_CORE * K       # 131072 gathered rows per core
ST_Q = 512                  # queries per supertile
N_ST = NQ_CORE // ST_Q      # 16
KW_ST = 2                   # supertiles per kw group
G_ST = ST_Q * K // 128      # 64 g-cols per supertile
ROW16 = 128                 # f16 units per table row (256B)

# ---------------------------------------------------------------------------
# walrus workaround: this nix walrus build supports ONE sync-wait per
# instruction; split extra waits onto NoOps inserted before the offender
# (same-engine program order preserves semantics). Also run
# codegen_inst_isa_subclasses (Bacc does; raw Bass doesn't) so extended
# instructions get their ISA bytes.
_orig_to_json_bytes = bass.Bass.to_json_bytes


def _fix_block(bb, ctr):
    insts = bb.get("instructions")
    if not isinstance(insts, list):
        return
    new = []
    for inst in insts:
        si = inst.get("sync_info")
        ow = si.get("on_wait") if isinstance(si, dict) else None
        if ow and len(ow) > 1:
            for w in ow[:-1]:
                ctr[0] += 1
                nop = {"engine": inst["engine"], "ins": [], "outs": [],
                       "name": f"I-wsplit-{ctr[0]}", "opcode": "NoOp",
                       "sync_info": {"on_update": [], "on_wait": [w]},
                       "text_hint": "wsplit"}
                if "debug" in inst:
                    nop["debug"] = inst["debug"]
                new.append(nop)
            si["on_wait"] = [ow[-1]]
        new.append(inst)
    bb["instructions"] = new


def _walk(o, ctr):
    if isinstance(o, dict):
        if isinstance(o.get("instructions"), list):
            _fix_block(o, ctr)
        for v in o.values():
            _walk(v, ctr)
    elif isinstance(o, list):
        for v in o:
            _walk(v, ctr)


def _to_json_bytes_split(self):
    mybir.codegen_inst_isa_subclasses(self)
    raw = _orig_to_json_bytes(self)
    d = json.loads(raw)
    ctr = [0]
    _walk(d, ctr)
    return json.dumps(d).encode()


bass.Bass.to_json_bytes = _to_json_bytes_split


def ap_view(t_ap, extra_offset, dims):
    """AP over tile t_ap with explicit free dims [[step, count], ...]
    (steps in elements); partition dim is taken from the tile."""
    return bass.AP(t_ap.tensor, t_ap.offset + extra_offset,
                   [t_ap.ap[0]] + list(dims))


def build_bass(kp, skip=()):
    global SKIP
    SKIP = set(skip)
    """kp: (15, 3) float32 numpy kernel points (runtime values baked)."""
    nc = bass.Bass(dynamic_dma_scratch_size=32768, num_swdge_queues=4)

    table_in = nc.dram_tensor("table", [M, ROW16], F16, kind="ExternalInput")
    qrep_in = nc.dram_tensor("qrep", [128, NK_CORE // 128, 3], F32,
                             kind="ExternalInput")
    idx_in = nc.dram_tensor("idx", [128, NK_CORE // 16], I16,
                            kind="ExternalInput")
    from concourse import library_config
    nc.gpsimd.load_library(library_config.mlp)
    w_in = nc.dram_tensor("w", [C_IN, P * C_OUT], F16, kind="ExternalInput")
    bias_in = nc.dram_tensor("bias", [C_OUT, 1], F32, kind="ExternalInput")
    mask120_in = nc.dram_tensor("mask120", [128, 120], F16, kind="ExternalInput")
    mask16_in = nc.dram_tensor("mask16", [128, 8], F16, kind="ExternalInput")
    ident_in = nc.dram_tensor("ident", [128, 128], F32, kind="ExternalInput")
    ones1_in = nc.dram_tensor("ones1", [1, 128], F16, kind="ExternalInput")
    kpb_in = nc.dram_tensor("kpb", [128, 48], F32, kind="ExternalInput")
    kpb3_in = nc.dram_tensor("kpb3", [128, 45], F16, kind="ExternalInput")
    onesc_in = nc.dram_tensor("onesc", [128, 1], F16, kind="ExternalInput")
    out_t = nc.dram_tensor("out", [NQ_CORE, C_OUT], F32, kind="ExternalOutput")

    with TileContext(nc) as tc:
        with tc.tile_pool(name="const", bufs=1) as cpool, \
             tc.tile_pool(name="gath", bufs=2) as gpool, \
             tc.tile_pool(name="kwp", bufs=2) as kwpool, \
             tc.tile_pool(name="kbd", bufs=1) as kbpool, \
             tc.tile_pool(name="wt", bufs=1) as wtpool, \
             tc.tile_pool(name="sm", bufs=2) as smpool, \
             tc.tile_pool(name="fin", bufs=2) as fpool, \
             tc.tile_pool(name="ps1", bufs=2, space="PSUM") as ps1pool, \
             tc.tile_pool(name="ps2", bufs=2, space="PSUM") as ps2pool, \
             tc.tile_pool(name="ps3", bufs=1, space="PSUM") as ps3pool:

            # ---- constants ----
            wp_t = cpool.tile([C_IN, P * C_OUT], F16, tag="wp")
            nc.sync.dma_start(wp_t[:], w_in[:])
            bias_t = cpool.tile([C_OUT, 1], F32, tag="bias")
            nc.sync.dma_start(bias_t[:], bias_in[:])
            mask120_t = cpool.tile([128, 120], F16, tag="m120")
            nc.sync.dma_start(mask120_t[:], mask120_in[:])
            mask16_t = cpool.tile([128, 8], F16, tag="m16")
            nc.sync.dma_start(mask16_t[:], mask16_in[:])
            ident_t = cpool.tile([128, 128], F32, tag="ident")
            nc.sync.dma_start(ident_t[:], ident_in[:])
            ones1_t = cpool.tile([1, 128], F16, tag="ones1")
            nc.sync.dma_start(ones1_t[:], ones1_in[:])
            kpb_t = cpool.tile([128, 48], F32, tag="kpb")
            nc.sync.dma_start(kpb_t[:], kpb_in[:])
            kpb3_t = cpool.tile([128, 45], F16, tag="kpb3")
            nc.sync.dma_start(kpb3_t[:], kpb3_in[:])
            onesc_t = cpool.tile([128, 1], F16, tag="onesc")
            nc.sync.dma_start(onesc_t[:], onesc_in[:])
            idx_all = cpool.tile([128, NK_CORE // 16], I16, tag="idxall")
            nc.sync.dma_start(idx_all[:], idx_in[:])
            qrep_all = cpool.tile([128, NK_CORE // 128, 3], F32, tag="qrall")
            nc.sync.dma_start(qrep_all[:], qrep_in[:])
            nidx_reg = nc.gpsimd.to_reg(1024)

            _main_pipeline(nc, tc, gpool, kwpool, kbpool, wtpool, smpool,
                           fpool, ps1pool, ps2pool, ps3pool, kp,
                           qrep_all, idx_all, out_t, table_in, wp_t, bias_t,
                           mask120_t, mask16_t, ident_t, ones1_t, kpb_t,
                           onesc_t, kpb3_t, nidx_reg)
    return nc


def _main_pipeline(nc, tc, gpool, kwpool, kbpool, wtpool, smpool, fpool,
                   ps1pool, ps2pool, ps3pool, kp, qrep_all, idx_all, out_t,
                   table_in, wp_t, bias_t, mask120_t, mask16_t, ident_t,
                   ones1_t, kpb_t, onesc_t, kpb3_t, nidx_reg):
    for kg in range(N_ST // KW_ST):  # kw group of 2 supertiles
        GQ = KW_ST * ST_Q            # 1024 queries
        GG = KW_ST * G_ST            # 128 g-cols
        gt = gpool.tile([128, GG, ROW16], F16, tag="gath")
        gt32 = gt[:].bitcast(F32)  # [128, GG, 64] f32 view
        # gathers: 16 chunks of 1024 idx
        if "gather" in SKIP:
            nc.vector.memset(gt[:], 0.0)
        for g in range(GG // 8):
            if "gather" in SKIP:
                break
            c0 = (kg * 16 + g) * 64
            nc.gpsimd.dma_gather(
                gt[:, g * 8:(g + 1) * 8, :], table_in[:],
                idx_all[:, c0:c0 + 64],
                1024, nidx_reg, ROW16, queue_num=g % 4)
        # rel = s - q (fp16), qrep sliced from resident tile
        rel = smpool.tile([128, GG, 3], F16, tag="rel")
        nc.vector.tensor_tensor(
            out=rel[:],
            in0=ap_view(gt32, 32, [[64, GG], [1, 3]]),
            in1=qrep_all[:, kg * GG:(kg + 1) * GG, :],
            op=mybir.AluOpType.subtract)
        # diff[g,p,d] = rel[g,d] - kp[p,d]; square; sum over d; sqrt; relu
        kwt = kwpool.tile([128, GG, P], F16, tag="kw")
        if "kw" in SKIP:
            nc.vector.memset(kwt[:], 0.0)
        else:
            diff = kwpool.tile([128, GG, P, 3], F16, tag="diff")
            nc.vector.tensor_tensor(
                out=diff[:],
                in0=ap_view(rel[:], 0, [[3, GG], [0, P], [1, 3]]),
                in1=ap_view(kpb3_t[:], 0, [[0, GG], [3, P], [1, 3]]),
                op=mybir.AluOpType.subtract)
            nc.scalar.activation(diff[:], diff[:],
                                 mybir.ActivationFunctionType.Square,
                                 bias=0.0, scale=1.0)
            d2 = kwpool.tile([128, GG, P], F16, tag="d2")
            with nc.allow_low_precision(reason="d2 sum of 3 sq in fp16"):
                nc.vector.tensor_reduce(out=d2[:], in_=diff[:],
                                        axis=mybir.AxisListType.X,
                                        op=mybir.AluOpType.add)
            # kw = relu(1 - sqrt(d2)/sigma) -> fp16
            nc.scalar.activation(d2[:], d2[:],
                                 mybir.ActivationFunctionType.Sqrt,
                                 bias=0.0, scale=1.0)
            nc.scalar.activation(kwt[:], d2[:],
                                 mybir.ActivationFunctionType.Relu,
                                 bias=1.0, scale=kpb_t[:, 46:47])

        for sti in range(KW_ST):
            st = kg * KW_ST + sti
            # kwbd (2 half-ST TT ops): [128, (bl32, q8, p15)] fp16
            kbd = kbpool.tile([128, 3840], F16, tag="kbd")
            kbd2 = kbpool.tile([128, 3840], F16, tag="kbd2")
            if "kwbd" in SKIP:
                nc.vector.memset(kbd[:], 0.0)
                nc.vector.memset(kbd2[:], 0.0)
            for hf, kb in ((0, kbd), (1, kbd2)) if "kwbd" not in SKIP else ():
                bl0 = sti * G_ST + hf * 32
                nc.vector.tensor_tensor(
                    out=ap_view(kb[:], 0,
                                [[120, 32], [15, 8], [1, 15]]),
                    in0=ap_view(kwt[:], bl0 * P,
                                [[P, 32], [0, 8], [1, P]]),
                    in1=ap_view(mask120_t[:], 0,
                                [[0, 32], [15, 8], [1, 15]]),
                    op=mybir.AluOpType.mult)
            # einsum1: 64 blocks
            wtt = wtpool.tile([64, 7680], F16, tag="wt")
            if "e1" in SKIP:
                nc.vector.memset(wtt[:], 0.0)
            for bg in range(16 if "e1" not in SKIP else 0):  # bank groups of 4 blocks (32 q)
                pse1 = ps1pool.tile([64, 480], F32, tag="pse1")
                for j in range(4):
                    bl = bg * 4 + j          # block in supertile
                    blg = sti * G_ST + bl    # g-col in group tile
                    kb = kbd if bl < 32 else kbd2
                    kbl = bl % 32
                    nc.tensor.matmul(
                        pse1[:, j * 120:(j + 1) * 120],
                        ap_view(gt[:], blg * ROW16, [[1, C_IN]]),
                        ap_view(kb[:], kbl * 120, [[1, 120]]),
                        start=True, stop=True)
                # evict (split DVE/ACT) -> fp16
                nc.vector.tensor_copy(
                    wtt[:, bg * 480:bg * 480 + 240],
                    pse1[:, 0:240])
                nc.scalar.copy(
                    wtt[:, bg * 480 + 240:bg * 480 + 480],
                    pse1[:, 240:480])
            # count row: zbd = z * mask16 -> ones-row matmul (fp16)
            zbd = smpool.tile([128, 512], F16, tag="zbd")
            nc.vector.tensor_tensor(
                out=zbd[:].rearrange("a (g j q) -> a g j q",
                                     g=16, j=4),
                in0=ap_view(gt32, (sti * G_ST) * 64 + 35,
                            [[256, 16], [64, 4], [0, 8]]),
                in1=ap_view(mask16_t[:], 0,
                            [[0, 16], [0, 4], [1, 8]]),
                op=mybir.AluOpType.mult)
            pscnt = ps3pool.tile([1, 512], F32, tag="pscnt")
            nc.tensor.matmul(pscnt[:], onesc_t[:], zbd[:],
                             start=True, stop=True)
            cntinv = smpool.tile([1, 512], F32, tag="cntinv")
            nc.vector.tensor_scalar(out=cntinv[:], in0=pscnt[:],
                                    scalar1=1.0, scalar2=None,
                                    op0=mybir.AluOpType.max)
            rscr = smpool.tile([1, 512], F32, tag="rscr")
            nc.vector.reciprocal_approx_accurate(out=cntinv[:], in_=cntinv[:],
                                                 scratch=rscr[:])
            cntinv16 = smpool.tile([1, 512], F16, tag="cntinv16")
            nc.vector.tensor_copy(cntinv16[:], cntinv[:])
            psrep = ps3pool.tile([128, 512], F32, tag="psrep")
            nc.tensor.matmul(psrep[:], ones1_t[:], cntinv16[:],
                             start=True, stop=True)
            cntrep = smpool.tile([128, 512], F32, tag="cntrep")
            nc.vector.tensor_copy(cntrep[:], psrep[:])

            # einsum2: out[o, s] accumulated over p (fp16 operands)
            pse2 = ps2pool.tile([128, 512], F32, tag="pse2")
            for p in range(P if "e2" not in SKIP else 1):
                nc.tensor.matmul(
                    pse2[:],
                    ap_view(wp_t[:], p * C_OUT, [[1, C_OUT]]),
                    ap_view(wtt[:], p,
                            [[480, 16], [120, 4], [15, 8]]),
                    start=(p == 0), stop=True)
            # divide by count, add bias
            e2sb = fpool.tile([128, 512], F32, tag="e2sb")
            nc.vector.tensor_tensor(out=e2sb[:], in0=pse2[:],
                                    in1=cntrep[:],
                                    op=mybir.AluOpType.mult)
            nc.vector.tensor_scalar(out=e2sb[:], in0=e2sb[:],
                                    scalar1=bias_t[:],
                                    scalar2=None,
                                    op0=mybir.AluOpType.add)
            # transpose 4x128 cols and store
            for t4 in range(4):
                pstr = ps3pool.tile([128, 128], F32, tag="pstr")
                nc.tensor.transpose(
                    pstr[:], e2sb[:, t4 * 128:(t4 + 1) * 128],
                    ident_t[:])
                trsb = fpool.tile([128, 128], F32, tag="trsb")
                nc.scalar.copy(trsb[:], pstr[:])
                # e2 cols are n-linear: plain contiguous store
                n0 = st * 512 + t4 * 128
                nc.sync.dma_start(out_t[n0:n0 + 128, :], trsb[:])


_BUILT = {}


def _get_nc(kp):
    key = kp.tobytes()
    if key not in _BUILT:
        _BUILT[key] = build_bass(kp)
    return _BUILT[key]


def _host_prep(query_points, support_points, support_features,
               neighbor_indices, weights, bias, kernel_points):
    qp = np.asarray(query_points, np.float32)
    sp = np.asarray(support_points, np.float32)
    sf = np.asarray(support_features, np.float32)
    ni = np.asarray(neighbor_indices)
    ni = np.clip(ni, 0, M - 1).astype(np.int16)
    w = np.ascontiguousarray(np.asarray(weights, np.float32))
    # w layout [C_IN, P*C_OUT] fp16: wl[c, p*C_OUT + o] = w[p, c, o]
    wl = np.ascontiguousarray(
        w.transpose(1, 0, 2).reshape(C_IN, P * C_OUT)).astype(np.float16)
    bias = np.asarray(bias, np.float32).reshape(C_OUT, 1)

    mask120 = np.zeros((128, 120), np.float16)
    for q in range(8):
        mask120[q * 16:(q + 1) * 16, q * 15:(q + 1) * 15] = 1.0
    mask16 = np.zeros((128, 8), np.float16)
    for q in range(8):
        mask16[q * 16:(q + 1) * 16, q] = 1.0
    ident = np.eye(128, dtype=np.float32)
    ones1 = np.ones((1, 128), np.float16)
    kpv = np.asarray(kernel_points, np.float32)
    kpb = np.zeros((128, 48), np.float32)
    for p in range(P):
        for d in range(3):
            kpb[:, 3 * p + d] = -kpv[p, d]
    kpb[:, 45] = 1e-10
    kpb[:, 46] = -1.0 / SIGMA
    kpb3 = np.tile(kpv.reshape(1, 45), (128, 1)).astype(np.float16)

    # host-built tables per batch: [M, ROW16] f16 rows (256B)
    # f16 cols 0..63 = feats; f32-view cols 32..34 = coords, 35 = z
    tables = []
    for b in range(B):
        t = np.zeros((M, ROW16), np.float16)
        tv32 = t.view(np.float32)  # [M, 64]
        t[:, 0:C_IN] = sf[b].astype(np.float16)
        tv32[:, 32:35] = sp[b]
        tv32[:, 35] = (np.abs(sf[b]).sum(axis=1) > 0).astype(np.float32)
        tables.append(t)

    in_maps = []
    for c in range(N_CORES):
        b, half = divmod(c, 2)
        n0 = half * NQ_CORE
        idx = ni[b, n0:n0 + NQ_CORE, :].reshape(NK_CORE)
        # chunk order: idx j in chunk -> partition j%128, col j//128;
        # idx tile wraps 16 partitions, replicated x8
        idx_l = idx.reshape(NK_CORE // 16, 16).T          # [16, NK/16]
        idx_l = np.ascontiguousarray(np.tile(idx_l, (8, 1)).astype(np.int16))
        qrep = np.repeat(qp[b, n0:n0 + NQ_CORE, :], K, axis=0)  # [NK, 3]
        qrep = qrep.reshape(NK_CORE // 128, 128, 3).transpose(1, 0, 2)
        qrep = np.ascontiguousarray(qrep)
        in_maps.append({
            "table": tables[b], "qrep": qrep, "idx": idx_l,
            "w": wl, "bias": bias, "mask120": mask120, "mask16": mask16,
            "ident": ident, "ones1": ones1, "kpb": kpb, "kpb3": kpb3,
            "onesc": np.ones((128, 1), np.float16),
        })
    return in_maps


def kernel(query_points, support_points, support_features, neighbor_indices,
           weights, bias, kernel_points):
    kp = np.asarray(kernel_points, np.float32)
    nc = _get_nc(kp)
    in_maps = _host_prep(query_points, support_points, support_features,
                         neighbor_indices, weights, bias, kernel_points)
    results = bass2jax.run_bass_via_pjrt(nc, in_maps, n_cores=N_CORES)
    out = np.zeros((B, N, C_OUT), np.float32)
    for c in range(N_CORES):
        b, half = divmod(c, 2)
        n0 = half * NQ_CORE
        out[b, n0:n0 + NQ_CORE, :] = np.asarray(results[c]["out"])
    return out


# revision 8
# speedup vs baseline: 1.3828x; 1.0732x over previous
"""KPConv (nn_KPConvFPN) Trainium2 Bass kernel, v3.

Sharding: 8 cores; core c handles batch b=c//2, query half (c%2)*8192.
Host packs a 256B/row table: [64 x fp16 feat | sx,sy,sz fp32 | z fp32 | pad].
Device pipeline per core:
  1. dma_gather (SWDGE, 4 queues) of the 16 neighbor rows per query:
     1024 idx/chunk, idx order (query-block, k) -> partitions (q8,k16).
  2. kw[n,k,p] = relu(1 - sqrt(|s - q - kp_p|^2)/sigma) via DVE/ACT,
     output fp16.
  3. einsum1 on PE per 8-query block: lhsT = gathered f (fp16), rhs =
     kw * blockdiag-mask (fp16) -> weightedT [c=64, (q',p)] in PSUM.
  4. einsum2 on PE in fp16: lhsT = W_p [64, 128] , rhs = weightedT
     strided -> out [o=128, n] accumulated over p.
  5. count[n] = sum_k z via masked-z ones-row matmul (fp16); fast
     reciprocal; divide + bias, PE-transpose to [n, o], store.
"""
import json
import math
import os

SKIP = set()

import numpy as np
import jax

import concourse.bass as bass
import concourse.mybir as mybir
from concourse.tile import TileContext
from concourse import bass2jax

F32 = mybir.dt.float32
F16 = mybir.dt.float16
I32 = mybir.dt.int32
I16 = mybir.dt.int16

B, N, M, K = 4, 16384, 16384, 16
C_IN, C_OUT, P = 64, 128, 15
SIGMA = 0.03
N_CORES = 8
NQ_CORE = N // 2            # 8192 queries per core
NK_CORE = NQ_CORE * K       # 131072 gathered rows per core
ST_Q = 512                  # queries per supertile
N_ST = NQ_CORE // ST_Q      # 16
KW_ST = 1                   # supertiles per kw group
G_ST = ST_Q * K // 128      # 64 g-cols per supertile
ROW16 = 128                 # f16 units per table row (256B)

# ---------------------------------------------------------------------------
# walrus workaround: this nix walrus build supports ONE sync-wait per
# instruction; split extra waits onto NoOps inserted before the offender
# (same-engine program order preserves semantics). Also run
# codegen_inst_isa_subclasses (Bacc does; raw Bass doesn't) so extended
# instructions get their ISA bytes.
_orig_to_json_bytes = bass.Bass.to_json_bytes


def _fix_block(bb, ctr):
    insts = bb.get("instructions")
    if not isinstance(insts, list):
        return
    new = []
    for inst in insts:
        si = inst.get("sync_info")
        ow = si.get("on_wait") if isinstance(si, dict) else None
        if ow and len(ow) > 1:
            for w in ow[:-1]:
                ctr[0] += 1
                nop = {"engine": inst["engine"], "ins": [], "outs": [],
                       "name": f"I-wsplit-{ctr[0]}", "opcode": "NoOp",
                       "sync_info": {"on_update": [], "on_wait": [w]},
                       "text_hint": "wsplit"}
                if "debug" in inst:
                    nop["debug"] = inst["debug"]
                new.append(nop)
            si["on_wait"] = [ow[-1]]
        new.append(inst)
    bb["instructions"] = new


def _walk(o, ctr):
    if isinstance(o, dict):
        if isinstance(o.get("instructions"), list):
            _fix_block(o, ctr)
        for v in o.values():
            _walk(v, ctr)
    elif isinstance(o, list):
        for v in o:
            _walk(v, ctr)


def _to_json_bytes_split(self):
    mybir.codegen_inst_isa_subclasses(self)
    raw = _orig_to_json_bytes(self)
    d = json.loads(raw)
    ctr = [0]
    _walk(d, ctr)
    return json.dumps(d).encode()


bass.Bass.to_json_bytes = _to_json_bytes_split


def ap_view(t_ap, extra_offset, dims):
    """AP over tile t_ap with explicit free dims [[step, count], ...]
    (steps in elements); partition dim is taken from the tile."""
    return bass.AP(t_ap.tensor, t_ap.offset + extra_offset,
                   [t_ap.ap[0]] + list(dims))


def build_bass(kp, skip=()):
    global SKIP
    SKIP = set(skip)
    """kp: (15, 3) float32 numpy kernel points (runtime values baked)."""
    nc = bass.Bass(dynamic_dma_scratch_size=32768, num_swdge_queues=4)

    table_in = nc.dram_tensor("table", [M, ROW16], F16, kind="ExternalInput")
    qrep_in = nc.dram_tensor("qrep", [128, NK_CORE // 128, 3], F32,
                             kind="ExternalInput")
    idx_in = nc.dram_tensor("idx", [128, NK_CORE // 16], I16,
                            kind="ExternalInput")
    from concourse import library_config
    nc.gpsimd.load_library(library_config.mlp)
    w_in = nc.dram_tensor("w", [C_IN, P * C_OUT], F16, kind="ExternalInput")
    bias_in = nc.dram_tensor("bias", [C_OUT, 1], F32, kind="ExternalInput")
    mask120_in = nc.dram_tensor("mask120", [128, 120], F16, kind="ExternalInput")
    mask16_in = nc.dram_tensor("mask16", [128, 8], F16, kind="ExternalInput")
    ident_in = nc.dram_tensor("ident", [128, 128], F32, kind="ExternalInput")
    ones1_in = nc.dram_tensor("ones1", [1, 128], F16, kind="ExternalInput")
    kpb_in = nc.dram_tensor("kpb", [128, 48], F32, kind="ExternalInput")
    kpb3_in = nc.dram_tensor("kpb3", [128, 45], F16, kind="ExternalInput")
    onesc_in = nc.dram_tensor("onesc", [128, 1], F16, kind="ExternalInput")
    out_t = nc.dram_tensor("out", [NQ_CORE, C_OUT], F32, kind="ExternalOutput")

    with TileContext(nc) as tc:
        with tc.tile_pool(name="const", bufs=1) as cpool, \
             tc.tile_pool(name="gath", bufs=2) as gpool, \
             tc.tile_pool(name="kwp", bufs=2) as kwpool, \
             tc.tile_pool(name="kbd", bufs=1) as kbpool, \
             tc.tile_pool(name="wt", bufs=1) as wtpool, \
             tc.tile_pool(name="sm", bufs=2) as smpool, \
             tc.tile_pool(name="fin", bufs=2) as fpool, \
             tc.tile_pool(name="ps1", bufs=2, space="PSUM") as ps1pool, \
             tc.tile_pool(name="ps2", bufs=2, space="PSUM") as ps2pool, \
             tc.tile_pool(name="ps3", bufs=1, space="PSUM") as ps3pool:

            # ---- constants ----
            wp_t = cpool.tile([C_IN, P * C_OUT], F16, tag="wp")
            nc.sync.dma_start(wp_t[:], w_in[:])
            bias_t = cpool.tile([C_OUT, 1], F32, tag="bias")
            nc.sync.dma_start(bias_t[:], bias_in[:])
            mask120_t = cpool.tile([128, 120], F16, tag="m120")
            nc.sync.dma_start(mask120_t[:], mask120_in[:])
            mask16_t = cpool.tile([128, 8], F16, tag="m16")
            nc.sync.dma_start(mask16_t[:], mask16_in[:])
            ident_t = cpool.tile([128, 128], F32, tag="ident")
            nc.sync.dma_start(ident_t[:], ident_in[:])
            ones1_t = cpool.tile([1, 128], F16, tag="ones1")
            nc.sync.dma_start(ones1_t[:], ones1_in[:])
            kpb_t = cpool.tile([128, 48], F32, tag="kpb")
            nc.sync.dma_start(kpb_t[:], kpb_in[:])
            kpb3_t = cpool.tile([128, 45], F16, tag="kpb3")
            nc.sync.dma_start(kpb3_t[:], kpb3_in[:])
            onesc_t = cpool.tile([128, 1], F16, tag="onesc")
            nc.sync.dma_start(onesc_t[:], onesc_in[:])
            idx_all = cpool.tile([128, NK_CORE // 16], I16, tag="idxall")
            nc.sync.dma_start(idx_all[:], idx_in[:])
            qrep_all = cpool.tile([128, NK_CORE // 128, 3], F32, tag="qrall")
            nc.sync.dma_start(qrep_all[:], qrep_in[:])
            nidx_reg = nc.gpsimd.to_reg(1024)

            _main_pipeline(nc, tc, gpool, kwpool, kbpool, wtpool, smpool,
                           fpool, ps1pool, ps2pool, ps3pool, kp,
                           qrep_all, idx_all, out_t, table_in, wp_t, bias_t,
                           mask120_t, mask16_t, ident_t, ones1_t, kpb_t,
                           onesc_t, kpb3_t, nidx_reg)
    return nc


def _main_pipeline(nc, tc, gpool, kwpool, kbpool, wtpool, smpool, fpool,
                   ps1pool, ps2pool, ps3pool, kp, qrep_all, idx_all, out_t,
                   table_in, wp_t, bias_t, mask120_t, mask16_t, ident_t,
                   ones1_t, kpb_t, onesc_t, kpb3_t, nidx_reg):
    for kg in range(N_ST // KW_ST):  # kw group of 2 supertiles
        GQ = KW_ST * ST_Q            # 1024 queries
        GG = KW_ST * G_ST            # 128 g-cols
        gt = gpool.tile([128, GG, ROW16], F16, tag="gath")
        gt32 = gt[:].bitcast(F32)  # [128, GG, 64] f32 view
        # gathers: 16 chunks of 1024 idx
        if "gather" in SKIP:
            nc.vector.memset(gt[:], 0.0)
        for g in range(GG // 8):
            if "gather" in SKIP:
                break
            c0 = (kg * (GG // 8) + g) * 64
            nc.gpsimd.dma_gather(
                gt[:, g * 8:(g + 1) * 8, :], table_in[:],
                idx_all[:, c0:c0 + 64],
                1024, nidx_reg, ROW16, queue_num=g % 4)
        # rel = s - q (fp16), qrep sliced from resident tile
        rel = smpool.tile([128, GG, 3], F16, tag="rel")
        nc.vector.tensor_tensor(
            out=rel[:],
            in0=ap_view(gt32, 32, [[64, GG], [1, 3]]),
            in1=qrep_all[:, kg * GG:(kg + 1) * GG, :],
            op=mybir.AluOpType.subtract)
        # diff[g,p,d] = rel[g,d] - kp[p,d]; square; sum over d; sqrt; relu
        kwt = kwpool.tile([128, GG, P], F16, tag="kw")
        if "kw" in SKIP:
            nc.vector.memset(kwt[:], 0.0)
        else:
            diff = kwpool.tile([128, GG, P, 3], F16, tag="diff")
            nc.vector.tensor_tensor(
                out=diff[:],
                in0=ap_view(rel[:], 0, [[3, GG], [0, P], [1, 3]]),
                in1=ap_view(kpb3_t[:], 0, [[0, GG], [3, P], [1, 3]]),
                op=mybir.AluOpType.subtract)
            nc.scalar.activation(diff[:], diff[:],
                                 mybir.ActivationFunctionType.Square,
                                 bias=0.0, scale=1.0)
            d2 = kwpool.tile([128, GG, P], F16, tag="d2")
            with nc.allow_low_precision(reason="d2 sum of 3 sq in fp16"):
                nc.vector.tensor_reduce(out=d2[:], in_=diff[:],
                                        axis=mybir.AxisListType.X,
                                        op=mybir.AluOpType.add)
            # kw = relu(1 - sqrt(d2)/sigma) -> fp16
            nc.scalar.activation(d2[:], d2[:],
                                 mybir.ActivationFunctionType.Sqrt,
                                 bias=0.0, scale=1.0)
            nc.scalar.activation(kwt[:], d2[:],
                                 mybir.ActivationFunctionType.Relu,
                                 bias=1.0, scale=kpb_t[:, 46:47])

        for sti in range(KW_ST):
            st = kg * KW_ST + sti
            # kwbd (2 half-ST TT ops): [128, (bl32, q8, p15)] fp16
            kbd = kbpool.tile([128, 3840], F16, tag="kbd")
            kbd2 = kbpool.tile([128, 3840], F16, tag="kbd2")
            if "kwbd" in SKIP:
                nc.vector.memset(kbd[:], 0.0)
                nc.vector.memset(kbd2[:], 0.0)
            for hf, kb in ((0, kbd), (1, kbd2)) if "kwbd" not in SKIP else ():
                bl0 = sti * G_ST + hf * 32
                nc.vector.tensor_tensor(
                    out=ap_view(kb[:], 0,
                                [[120, 32], [15, 8], [1, 15]]),
                    in0=ap_view(kwt[:], bl0 * P,
                                [[P, 32], [0, 8], [1, P]]),
                    in1=ap_view(mask120_t[:], 0,
                                [[0, 32], [15, 8], [1, 15]]),
                    op=mybir.AluOpType.mult)
            # einsum1: 64 blocks
            wtt = wtpool.tile([64, 7680], F16, tag="wt")
            if "e1" in SKIP:
                nc.vector.memset(wtt[:], 0.0)
            for bg in range(16 if "e1" not in SKIP else 0):  # bank groups of 4 blocks (32 q)
                pse1 = ps1pool.tile([64, 480], F32, tag="pse1")
                for j in range(4):
                    bl = bg * 4 + j          # block in supertile
                    blg = sti * G_ST + bl    # g-col in group tile
                    kb = kbd if bl < 32 else kbd2
                    kbl = bl % 32
                    nc.tensor.matmul(
                        pse1[:, j * 120:(j + 1) * 120],
                        ap_view(gt[:], blg * ROW16, [[1, C_IN]]),
                        ap_view(kb[:], kbl * 120, [[1, 120]]),
                        start=True, stop=True)
                # evict (split DVE/ACT) -> fp16
                nc.vector.tensor_copy(
                    wtt[:, bg * 480:bg * 480 + 240],
                    pse1[:, 0:240])
                nc.scalar.copy(
                    wtt[:, bg * 480 + 240:bg * 480 + 480],
                    pse1[:, 240:480])
            # count row: zbd = z * mask16 -> ones-row matmul (fp16)
            zbd = smpool.tile([128, 512], F16, tag="zbd")
            nc.vector.tensor_tensor(
                out=zbd[:].rearrange("a (g j q) -> a g j q",
                                     g=16, j=4),
                in0=ap_view(gt32, (sti * G_ST) * 64 + 35,
                            [[256, 16], [64, 4], [0, 8]]),
                in1=ap_view(mask16_t[:], 0,
                            [[0, 16], [0, 4], [1, 8]]),
                op=mybir.AluOpType.mult)
            pscnt = ps3pool.tile([1, 512], F32, tag="pscnt")
            nc.tensor.matmul(pscnt[:], onesc_t[:], zbd[:],
                             start=True, stop=True)
            cntinv = smpool.tile([1, 512], F32, tag="cntinv")
            nc.vector.tensor_scalar(out=cntinv[:], in0=pscnt[:],
                                    scalar1=1.0, scalar2=None,
                                    op0=mybir.AluOpType.max)
            rscr = smpool.tile([1, 512], F32, tag="rscr")
            nc.vector.reciprocal_approx_accurate(out=cntinv[:], in_=cntinv[:],
                                                 scratch=rscr[:])
            cntinv16 = smpool.tile([1, 512], F16, tag="cntinv16")
            nc.vector.tensor_copy(cntinv16[:], cntinv[:])
            psrep = ps3pool.tile([128, 512], F32, tag="psrep")
            nc.tensor.matmul(psrep[:], ones1_t[:], cntinv16[:],
                             start=True, stop=True)
            cntrep = smpool.tile([128, 512], F32, tag="cntrep")
            nc.vector.tensor_copy(cntrep[:], psrep[:])

            # einsum2: out[o, s] accumulated over p (fp16 operands)
            pse2 = ps2pool.tile([128, 512], F32, tag="pse2")
            for p in range(P if "e2" not in SKIP else 1):
                nc.tensor.matmul(
                    pse2[:],
                    ap_view(wp_t[:], p * C_OUT, [[1, C_OUT]]),
                    ap_view(wtt[:], p,
                            [[480, 16], [120, 4], [15, 8]]),
                    start=(p == 0), stop=True)
            # divide by count, add bias
            e2sb = fpool.tile([128, 512], F32, tag="e2sb")
            nc.vector.tensor_tensor(out=e2sb[:], in0=pse2[:],
                                    in1=cntrep[:],
                                    op=mybir.AluOpType.mult)
            nc.vector.tensor_scalar(out=e2sb[:], in0=e2sb[:],
                                    scalar1=bias_t[:],
                                    scalar2=None,
                                    op0=mybir.AluOpType.add)
            # transpose 4x128 cols and store
            for t4 in range(4):
                pstr = ps3pool.tile([128, 128], F32, tag="pstr")
                nc.tensor.transpose(
                    pstr[:], e2sb[:, t4 * 128:(t4 + 1) * 128],
                    ident_t[:])
                trsb = fpool.tile([128, 128], F32, tag="trsb")
                nc.scalar.copy(trsb[:], pstr[:])
                # e2 cols are n-linear: plain contiguous store
                n0 = st * 512 + t4 * 128
                nc.sync.dma_start(out_t[n0:n0 + 128, :], trsb[:])


_BUILT = {}


def _get_nc(kp):
    key = kp.tobytes()
    if key not in _BUILT:
        _BUILT[key] = build_bass(kp)
    return _BUILT[key]


def _host_prep(query_points, support_points, support_features,
               neighbor_indices, weights, bias, kernel_points):
    qp = np.asarray(query_points, np.float32)
    sp = np.asarray(support_points, np.float32)
    sf = np.asarray(support_features, np.float32)
    ni = np.asarray(neighbor_indices)
    ni = np.clip(ni, 0, M - 1).astype(np.int16)
    w = np.ascontiguousarray(np.asarray(weights, np.float32))
    # w layout [C_IN, P*C_OUT] fp16: wl[c, p*C_OUT + o] = w[p, c, o]
    wl = np.ascontiguousarray(
        w.transpose(1, 0, 2).reshape(C_IN, P * C_OUT)).astype(np.float16)
    bias = np.asarray(bias, np.float32).reshape(C_OUT, 1)

    mask120 = np.zeros((128, 120), np.float16)
    for q in range(8):
        mask120[q * 16:(q + 1) * 16, q * 15:(q + 1) * 15] = 1.0
    mask16 = np.zeros((128, 8), np.float16)
    for q in range(8):
        mask16[q * 16:(q + 1) * 16, q] = 1.0
    ident = np.eye(128, dtype=np.float32)
    ones1 = np.ones((1, 128), np.float16)
    kpv = np.asarray(kernel_points, np.float32)
    kpb = np.zeros((128, 48), np.float32)
    for p in range(P):
        for d in range(3):
            kpb[:, 3 * p + d] = -kpv[p, d]
    kpb[:, 45] = 1e-10
    kpb[:, 46] = -1.0 / SIGMA
    kpb3 = np.tile(kpv.reshape(1, 45), (128, 1)).astype(np.float16)

    # host-built tables per batch: [M, ROW16] f16 rows (256B)
    # f16 cols 0..63 = feats; f32-view cols 32..34 = coords, 35 = z
    tables = []
    for b in range(B):
        t = np.zeros((M, ROW16), np.float16)
        tv32 = t.view(np.float32)  # [M, 64]
        t[:, 0:C_IN] = sf[b].astype(np.float16)
        tv32[:, 32:35] = sp[b]
        tv32[:, 35] = (np.abs(sf[b]).sum(axis=1) > 0).astype(np.float32)
        tables.append(t)

    in_maps = []
    for c in range(N_CORES):
        b, half = divmod(c, 2)
        n0 = half * NQ_CORE
        idx = ni[b, n0:n0 + NQ_CORE, :].reshape(NK_CORE)
        # chunk order: idx j in chunk -> partition j%128, col j//128;
        # idx tile wraps 16 partitions, replicated x8
        idx_l = idx.reshape(NK_CORE // 16, 16).T          # [16, NK/16]
        idx_l = np.ascontiguousarray(np.tile(idx_l, (8, 1)).astype(np.int16))
        qrep = np.repeat(qp[b, n0:n0 + NQ_CORE, :], K, axis=0)  # [NK, 3]
        qrep = qrep.reshape(NK_CORE // 128, 128, 3).transpose(1, 0, 2)
        qrep = np.ascontiguousarray(qrep)
        in_maps.append({
            "table": tables[b], "qrep": qrep, "idx": idx_l,
            "w": wl, "bias": bias, "mask120": mask120, "mask16": mask16,
            "ident": ident, "ones1": ones1, "kpb": kpb, "kpb3": kpb3,
            "onesc": np.ones((128, 1), np.float16),
        })
    return in_maps


def kernel(query_points, support_points, support_features, neighbor_indices,
           weights, bias, kernel_points):
    kp = np.asarray(kernel_points, np.float32)
    nc = _get_nc(kp)
    in_maps = _host_prep(query_points, support_points, support_features,
                         neighbor_indices, weights, bias, kernel_points)
    results = bass2jax.run_bass_via_pjrt(nc, in_maps, n_cores=N_CORES)
    out = np.zeros((B, N, C_OUT), np.float32)
    for c in range(N_CORES):
        b, half = divmod(c, 2)
        n0 = half * NQ_CORE
        out[b, n0:n0 + NQ_CORE, :] = np.asarray(results[c]["out"])
    return out
